# revision 3
# baseline (speedup 1.0000x reference)
"""MixedFeatureEmbedder Trainium2 kernel.

Data-parallel over 8 NeuronCores: each core handles 1024 batch rows.

Categorical half: indices are computed on DVE directly in the SWDGE
"wrapped" layout (int16 at [k%16, k//16], replicated across the 8 Q7
core groups) with global ordering k = f*1024 + b. Eight
InstDMAGatherAnt calls (4 features x 1024 batches = 4096 rows of 512B
each) gather from HBM; strided 3D DMAs write each feature column to
the output's odd feature slots.

Numeric half: PE transpose of x's even columns + K=33 matmul against a
block-diagonal [W; b] matrix -> x*W + b in PSUM, evacuated by the
scalar engine, then DMA'd to the even feature slots.
"""

import numpy as np

import concourse.bacc as bacc
import concourse.bass as bass
import concourse.mybir as mybir
import concourse.tile as tile
from concourse.bass_utils import run_bass_kernel_spmd
from concourse.masks import make_identity

N_CORES = 8
BATCH = 8192
B_SHARD = BATCH // N_CORES  # 1024
NF = 64
NNUM = 32
NCAT = 32
CARD = 100
D = 128
P = 128
TILES = B_SHARD // P  # 8
FPG = 4  # features per gather
GATHERS = NCAT // FPG  # 8
KPG = FPG * B_SHARD  # rows per gather = 4096
WRAP = KPG // 16  # wrapped idx cols per gather = 256
C_RINT = float(2**23)  # (x + 2^23) - 2^23 == rint(x) in f32

f32 = mybir.dt.float32
i16 = mybir.dt.int16
Alu = mybir.AluOpType


def _kernel_body(tc, out, x, w, bnum, emb):
    nc = tc.nc
    emb_flat = emb.rearrange("t c d -> (t c) d")
    # out viewed as (p, btile, feat, d): b = btile*128 + p
    outv = out.rearrange("(bt p) f d -> p bt f d", p=P)

    with (
        tc.tile_pool(name="const", bufs=1) as cpool,
        tc.tile_pool(name="work", bufs=3) as wpool,
        tc.tile_pool(name="gat", bufs=3) as gpool,
        tc.tile_pool(name="nbf", bufs=2) as npool,
        tc.tile_pool(name="ps", bufs=4, space="PSUM") as pspool,
        tc.tile_pool(name="pst", bufs=2, space="PSUM") as pstpool,
    ):
        # ---- constants ----
        identity = cpool.tile([P, P], f32)
        make_identity(nc, identity)

        # wrapped-layout per-feature offsets: col = f*64 + bd -> f*CARD
        offs16 = cpool.tile([P, NCAT * 64], i16)
        nc.gpsimd.iota(
            offs16.rearrange("p (f bd) -> p f bd", f=NCAT),
            pattern=[[CARD, NCAT], [0, 64]],
            base=0,
            channel_multiplier=0,
        )

        # block-diagonal [W; ones-row bias] matrix: (33, 32*128)
        WB = cpool.tile([NNUM + 1, NNUM * D], f32)
        nc.vector.memset(WB[0:NNUM, :], 0.0)
        nc.sync.dma_start(
            out=WB[NNUM : NNUM + 1, :], in_=bnum.rearrange("f d -> (f d)")
        )
        for f in range(NNUM):
            nc.sync.dma_start(
                out=WB[f : f + 1, f * D : (f + 1) * D], in_=w[f : f + 1, :]
            )

        # whole x shard resident: (128, 8 tiles * 64 feats) for numeric path
        xall = cpool.tile([P, TILES * NF], f32)
        nc.sync.dma_start(
            out=xall.rearrange("p (t f) -> p t f", f=NF),
            in_=x.rearrange("(t p) f -> p t f", p=P),
        )

        # ---- categorical indices, wrapped layout, all features ----
        # k = f*1024 + b; b = bd*16 + bm  ->  idx16[g*16+bm, f*64+bd]
        xw = cpool.tile([P, NCAT * 64], f32)
        xcat = x.rearrange("(bd bm) (f two) -> bm f bd two", bm=16, two=2)[
            :, :, :, 1
        ]  # (16, 32, 64) strided DRAM view of cat features
        for g in range(8):
            nc.sync.dma_start(
                out=xw[g * 16 : (g + 1) * 16].rearrange(
                    "bm (f bd) -> bm f bd", f=NCAT
                ),
                in_=xcat,
            )
        nc.vector.tensor_scalar(
            out=xw, in0=xw, scalar1=C_RINT, scalar2=C_RINT,
            op0=Alu.add, op1=Alu.subtract,
        )
        nc.vector.tensor_scalar(
            out=xw, in0=xw, scalar1=float(CARD - 1), scalar2=0.0,
            op0=Alu.min, op1=Alu.max,
        )
        idx16 = cpool.tile([P, NCAT * 64], i16)
        nc.vector.tensor_copy(out=idx16, in_=xw)
        nc.vector.tensor_tensor(out=idx16, in0=idx16, in1=offs16, op=Alu.add)

        # ---- categorical gathers: 4 features x 1024 batches each ----
        for fg in range(GATHERS):
            gbuf = gpool.tile([P, KPG // P, D], f32, name="gbuf")
            nc.gpsimd.dma_gather(
                out_ap=gbuf[:],
                in_ap=emb_flat,
                idxs_ap=idx16[:, fg * WRAP : (fg + 1) * WRAP],
                num_idxs=KPG,
                num_idxs_reg=KPG,
                elem_size=D,
                single_packet=False,
            )
            # gbuf[p, f_loc*8 + bt, :] = emb row for (b = bt*128+p,
            # f = fg*4+f_loc); write each feature column out
            for fl in range(FPG):
                slot = 2 * (fg * FPG + fl) + 1
                nc.scalar.dma_start(
                    out=outv[:, :, slot, :],
                    in_=gbuf[:, fl * TILES : (fl + 1) * TILES, :],
                )

        # ---- numeric per 128-row tile ----
        for t in range(TILES):
            xt_num = xall[:, t * NF : (t + 1) * NF : 2]  # (128, 32)
            xnT_ps = pstpool.tile([NNUM, P], f32, name="xnT_ps", space="PSUM")
            nc.tensor.transpose(out=xnT_ps, in_=xt_num, identity=identity)
            aug = wpool.tile([NNUM + 1, P], f32, name="aug")
            nc.vector.tensor_copy(out=aug[0:NNUM, :], in_=xnT_ps)
            nc.vector.memset(aug[NNUM : NNUM + 1, :], 1.0)

            nbuf = npool.tile([P, NNUM * D], f32, name="nbuf")
            for g in range(NNUM * D // 512):  # 8 groups of 4 features
                ps = pspool.tile([P, 512], f32, name="ps", space="PSUM")
                nc.tensor.matmul(
                    out=ps,
                    lhsT=aug,
                    rhs=WB[:, g * 512 : (g + 1) * 512],
                    start=True,
                    stop=True,
                )
                nc.scalar.copy(out=nbuf[:, g * 512 : (g + 1) * 512], in_=ps)

            nc.sync.dma_start(
                out=out[t * P : (t + 1) * P, 0::2, :],
                in_=nbuf.rearrange("p (f d) -> p f d", d=D),
            )


_NC_CACHE = None


def _build():
    global _NC_CACHE
    if _NC_CACHE is not None:
        return _NC_CACHE
    nc = bacc.Bacc(
        "TRN2", target_bir_lowering=False, debug=False, num_devices=N_CORES
    )
    x = nc.dram_tensor("x", (B_SHARD, NF), f32, kind="ExternalInput").ap()
    w = nc.dram_tensor("W_num", (NNUM, D), f32, kind="ExternalInput").ap()
    bnum = nc.dram_tensor("b_num", (NNUM, D), f32, kind="ExternalInput").ap()
    emb = nc.dram_tensor("emb_tables", (NCAT, CARD, D), f32, kind="ExternalInput").ap()
    out = nc.dram_tensor("out", (B_SHARD, NF, D), f32, kind="ExternalOutput").ap()
    with tile.TileContext(nc) as tc:
        _kernel_body(tc, out, x, w, bnum, emb)
    nc.compile()
    _NC_CACHE = nc
    return nc


def _run(inputs, **kwargs):
    nc = _build()
    x = np.ascontiguousarray(np.asarray(inputs["x"], dtype=np.float32))
    w = np.ascontiguousarray(np.asarray(inputs["W_num"], dtype=np.float32))
    b = np.ascontiguousarray(np.asarray(inputs["b_num"], dtype=np.float32))
    emb = np.ascontiguousarray(np.asarray(inputs["emb_tables"], dtype=np.float32))
    in_maps = [
        {
            "x": np.ascontiguousarray(x[i * B_SHARD : (i + 1) * B_SHARD]),
            "W_num": w,
            "b_num": b,
            "emb_tables": emb,
        }
        for i in range(N_CORES)
    ]
    res = run_bass_kernel_spmd(nc, in_maps, core_ids=list(range(N_CORES)), **kwargs)
    full = np.concatenate([r["out"] for r in res.results], axis=0)
    return full, res


def kernel(x, W_num, b_num, emb_tables):
    full, _ = _run(
        {"x": x, "W_num": W_num, "b_num": b_num, "emb_tables": emb_tables}
    )
    return full


# revision 5
# speedup vs baseline: 2.3133x; 2.3133x over previous
"""MixedFeatureEmbedder Trainium2 kernel (one-hot matmul gather).

Data-parallel over 8 NeuronCores: each core handles 1024 batch rows.

Categorical half (no DMA gather — all PE):
  idx = clip(rint(x_cat), 0, 99) on DVE; PE-transpose idx columns to
  rows; broadcast each feature's idx row across 100 partitions with a
  selector matmul (bf16, exact for small ints); build the one-hot via
  DVE is_equal against the partition index; then out = onehot.T @
  table[f] on PE (fp32) and evacuate PSUM via the scalar engine.

Numeric half: PE transpose of x's even columns + K=33 matmul against a
block-diagonal [W; b] matrix -> x*W + b in PSUM, scalar-engine evac.
"""

import numpy as np

import concourse.bacc as bacc
import concourse.bass as bass
import concourse.mybir as mybir
import concourse.tile as tile
from concourse.bass_utils import run_bass_kernel_spmd
from concourse.masks import make_identity

N_CORES = 8
BATCH = 8192
B_SHARD = BATCH // N_CORES  # 1024
NF = 64
NNUM = 32
NCAT = 32
CARD = 100
D = 128
P = 128
TILES = B_SHARD // P  # 8
TPC = 4  # tiles per chunk
CHUNKS = TILES // TPC  # 2
NB = TPC * P  # batch per chunk = 512
C_RINT = float(2**23)  # (x + 2^23) - 2^23 == rint(x) in f32

f32 = mybir.dt.float32
bf16 = mybir.dt.bfloat16
i32 = mybir.dt.int32
Alu = mybir.AluOpType


def _kernel_body(tc, out, x, w, bnum, emb):
    nc = tc.nc

    with (
        tc.tile_pool(name="const", bufs=1) as cpool,
        tc.tile_pool(name="work", bufs=3) as wpool,
        tc.tile_pool(name="oh", bufs=6) as ohpool,
        tc.tile_pool(name="cb", bufs=3) as cbpool,
        tc.tile_pool(name="nbf", bufs=2) as npool,
        tc.tile_pool(name="pst", bufs=2, space="PSUM") as pstpool,
        tc.tile_pool(name="psb", bufs=2, space="PSUM") as psbpool,
        tc.tile_pool(name="psn", bufs=2, space="PSUM") as psnpool,
        tc.tile_pool(name="psg", bufs=2, space="PSUM") as psgpool,
    ):
        # ---- constants ----
        identity = cpool.tile([P, P], f32)
        make_identity(nc, identity)

        # iota100[p, 0] = p (f32) for the one-hot compare
        iota_i = cpool.tile([P, 1], i32)
        nc.gpsimd.iota(iota_i, pattern=[[0, 1]], base=0, channel_multiplier=1)
        iota100 = cpool.tile([P, 1], f32)
        nc.vector.tensor_copy(out=iota100, in_=iota_i)

        # selector: SEL[k, f*CARD + m] = (k == f), bf16
        SEL = cpool.tile([NCAT, NCAT * CARD], bf16)
        nc.gpsimd.memset(SEL, 0.0)
        nc.gpsimd.affine_select(
            out=SEL,
            in_=SEL,
            compare_op=Alu.not_equal,
            fill=1.0,
            base=0,
            pattern=[[1, NCAT], [0, CARD]],
            channel_multiplier=-1,
        )

        # tables resident in SBUF: tablesSB[c, f*D + d] = emb[f, c, d]
        tablesSB = cpool.tile([CARD, NCAT * D], f32)
        nc.sync.dma_start(
            out=tablesSB.rearrange("c (f d) -> c f d", d=D),
            in_=emb.rearrange("f c d -> c f d"),
        )

        # block-diagonal [W; ones-row bias] matrix: (33, 32*128)
        WB = cpool.tile([NNUM + 1, NNUM * D], f32)
        nc.vector.memset(WB[0:NNUM, :], 0.0)
        nc.sync.dma_start(
            out=WB[NNUM : NNUM + 1, :], in_=bnum.rearrange("f d -> (f d)")
        )
        for f in range(NNUM):
            nc.sync.dma_start(
                out=WB[f : f + 1, f * D : (f + 1) * D], in_=w[f : f + 1, :]
            )

        # whole x shard resident: (128, 8 tiles * 64 feats)
        xall = cpool.tile([P, TILES * NF], f32)
        nc.sync.dma_start(
            out=xall.rearrange("p (t f) -> p t f", f=NF),
            in_=x.rearrange("(t p) f -> p t f", p=P),
        )

        for c in range(CHUNKS):
            # ---- per-tile: idx prep, transposes, numeric ----
            psum_xc = pstpool.tile([NCAT, NB], f32, name="psum_xc", tag="pst", space="PSUM")
            for tl in range(TPC):
                t = c * TPC + tl
                # categorical indices for this tile
                idx_f = wpool.tile([P, NCAT], f32, name="idx_f")
                nc.vector.tensor_scalar(
                    out=idx_f, in0=xall[:, t * NF + 1 : (t + 1) * NF : 2],
                    scalar1=C_RINT, scalar2=C_RINT,
                    op0=Alu.add, op1=Alu.subtract,
                )
                nc.vector.tensor_scalar(
                    out=idx_f, in0=idx_f, scalar1=float(CARD - 1), scalar2=0.0,
                    op0=Alu.min, op1=Alu.max,
                )
                nc.tensor.transpose(
                    out=psum_xc[:, tl * P : (tl + 1) * P],
                    in_=idx_f,
                    identity=identity,
                )

                # numeric: x^T, aug, K=33 matmuls against WB
                psum_xn = pstpool.tile(
                    [NNUM, P], f32, name="psum_xn", tag="pst", space="PSUM"
                )
                nc.tensor.transpose(
                    out=psum_xn,
                    in_=xall[:, t * NF : (t + 1) * NF : 2],
                    identity=identity,
                )
                aug = wpool.tile([NNUM + 1, P], f32, name="aug")
                nc.vector.tensor_copy(out=aug[0:NNUM, :], in_=psum_xn)
                nc.vector.memset(aug[NNUM : NNUM + 1, :], 1.0)

                nbuf = npool.tile([P, NNUM * D], f32, name="nbuf")
                for g in range(NNUM * D // 512):
                    ps = psnpool.tile([P, 512], f32, name="ps", space="PSUM")
                    nc.tensor.matmul(
                        out=ps,
                        lhsT=aug,
                        rhs=WB[:, g * 512 : (g + 1) * 512],
                        start=True,
                        stop=True,
                    )
                    nc.scalar.copy(out=nbuf[:, g * 512 : (g + 1) * 512], in_=ps)
                nc.sync.dma_start(
                    out=out[t * P : (t + 1) * P, 0::2, :],
                    in_=nbuf.rearrange("p (f d) -> p f d", d=D),
                )

            # idx rows for the whole chunk, bf16 (exact for ints < 256)
            xidxT = wpool.tile([NCAT, NB], bf16, name="xidxT")
            nc.vector.tensor_copy(out=xidxT, in_=psum_xc)

            # ---- categorical: one-hot matmul gather ----
            for fg in range(NCAT // 4):
                onehots = []
                for fl in range(4):
                    f = fg * 4 + fl
                    ps_bc = psbpool.tile(
                        [CARD, NB], f32, name="ps_bc", space="PSUM"
                    )
                    nc.tensor.matmul(
                        out=ps_bc,
                        lhsT=SEL[:, f * CARD : (f + 1) * CARD],
                        rhs=xidxT,
                        start=True,
                        stop=True,
                    )
                    oh = ohpool.tile([CARD, NB], f32, name="oh")
                    nc.vector.tensor_scalar(
                        out=oh, in0=ps_bc, scalar1=iota100[0:CARD, :],
                        scalar2=None, op0=Alu.is_equal,
                    )
                    onehots.append(oh)
                for tl in range(TPC):
                    t = c * TPC + tl
                    ps_g = psgpool.tile([P, 512], f32, name="ps_g", space="PSUM")
                    for fl in range(4):
                        f = fg * 4 + fl
                        nc.tensor.matmul(
                            out=ps_g[:, fl * D : (fl + 1) * D],
                            lhsT=onehots[fl][:, tl * P : (tl + 1) * P],
                            rhs=tablesSB[:, f * D : (f + 1) * D],
                            start=True,
                            stop=True,
                        )
                    cbuf = cbpool.tile([P, 512], f32, name="cbuf")
                    nc.scalar.copy(out=cbuf, in_=ps_g)
                    nc.sync.dma_start(
                        out=out[
                            t * P : (t + 1) * P, 8 * fg + 1 : 8 * fg + 8 : 2, :
                        ],
                        in_=cbuf.rearrange("p (f d) -> p f d", d=D),
                    )


_NC_CACHE = None


def _build():
    global _NC_CACHE
    if _NC_CACHE is not None:
        return _NC_CACHE
    nc = bacc.Bacc(
        "TRN2", target_bir_lowering=False, debug=False, num_devices=N_CORES
    )
    x = nc.dram_tensor("x", (B_SHARD, NF), f32, kind="ExternalInput").ap()
    w = nc.dram_tensor("W_num", (NNUM, D), f32, kind="ExternalInput").ap()
    bnum = nc.dram_tensor("b_num", (NNUM, D), f32, kind="ExternalInput").ap()
    emb = nc.dram_tensor("emb_tables", (NCAT, CARD, D), f32, kind="ExternalInput").ap()
    out = nc.dram_tensor("out", (B_SHARD, NF, D), f32, kind="ExternalOutput").ap()
    with tile.TileContext(nc) as tc:
        _kernel_body(tc, out, x, w, bnum, emb)
    nc.compile()
    _NC_CACHE = nc
    return nc


def _run(inputs, **kwargs):
    nc = _build()
    x = np.ascontiguousarray(np.asarray(inputs["x"], dtype=np.float32))
    w = np.ascontiguousarray(np.asarray(inputs["W_num"], dtype=np.float32))
    b = np.ascontiguousarray(np.asarray(inputs["b_num"], dtype=np.float32))
    emb = np.ascontiguousarray(np.asarray(inputs["emb_tables"], dtype=np.float32))
    in_maps = [
        {
            "x": np.ascontiguousarray(x[i * B_SHARD : (i + 1) * B_SHARD]),
            "W_num": w,
            "b_num": b,
            "emb_tables": emb,
        }
        for i in range(N_CORES)
    ]
    res = run_bass_kernel_spmd(nc, in_maps, core_ids=list(range(N_CORES)), **kwargs)
    full = np.concatenate([r["out"] for r in res.results], axis=0)
    return full, res


def kernel(x, W_num, b_num, emb_tables):
    full, _ = _run(
        {"x": x, "W_num": W_num, "b_num": b_num, "emb_tables": emb_tables}
    )
    return full


# revision 6
# speedup vs baseline: 2.6005x; 1.1241x over previous
"""MixedFeatureEmbedder Trainium2 kernel (one-hot matmul gather).

Data-parallel over 8 NeuronCores: each core handles 1024 batch rows.

Categorical half (no DMA gather — all PE):
  idx = clip(rint(x_cat), 0, 99) on DVE; PE-transpose idx columns to
  rows; broadcast each feature's idx row across 100 partitions with a
  selector matmul (bf16, exact for small ints); build the one-hot via
  DVE is_equal against the partition index; then out = onehot.T @
  table[f] on PE (fp32) and evacuate PSUM via the scalar engine.

Numeric half: PE transpose of x's even columns + K=33 matmul against a
block-diagonal [W; b] matrix -> x*W + b in PSUM, scalar-engine evac.
"""

import numpy as np

import concourse.bacc as bacc
import concourse.bass as bass
import concourse.mybir as mybir
import concourse.tile as tile
from concourse.bass_utils import run_bass_kernel_spmd
from concourse.masks import make_identity

N_CORES = 8
BATCH = 8192
B_SHARD = BATCH // N_CORES  # 1024
NF = 64
NNUM = 32
NCAT = 32
CARD = 100
D = 128
P = 128
TILES = B_SHARD // P  # 8
TPC = 4  # tiles per chunk
CHUNKS = TILES // TPC  # 2
NB = TPC * P  # batch per chunk = 512
C_RINT = float(2**23)  # (x + 2^23) - 2^23 == rint(x) in f32

f32 = mybir.dt.float32
bf16 = mybir.dt.bfloat16
f16 = mybir.dt.float16
i32 = mybir.dt.int32
Alu = mybir.AluOpType


def _kernel_body(tc, out, x, w, bnum, emb):
    nc = tc.nc

    with (
        tc.tile_pool(name="const", bufs=1) as cpool,
        tc.tile_pool(name="work", bufs=3) as wpool,
        tc.tile_pool(name="oh", bufs=6) as ohpool,
        tc.tile_pool(name="cb", bufs=3) as cbpool,
        tc.tile_pool(name="nbf", bufs=2) as npool,
        tc.tile_pool(name="pst", bufs=2, space="PSUM") as pstpool,
        tc.tile_pool(name="psb", bufs=2, space="PSUM") as psbpool,
        tc.tile_pool(name="psn", bufs=2, space="PSUM") as psnpool,
        tc.tile_pool(name="psg", bufs=2, space="PSUM") as psgpool,
    ):
        # ---- constants ----
        identity = cpool.tile([P, P], f32)
        make_identity(nc, identity)

        # iota100[p, 0] = p (f32) for the one-hot compare
        iota_i = cpool.tile([P, 1], i32)
        nc.gpsimd.iota(iota_i, pattern=[[0, 1]], base=0, channel_multiplier=1)
        iota100 = cpool.tile([P, 1], f32)
        nc.vector.tensor_copy(out=iota100, in_=iota_i)

        # selector: SEL[k, f*CARD + m] = (k == f), bf16
        SEL = cpool.tile([NCAT, NCAT * CARD], bf16)
        nc.gpsimd.memset(SEL, 0.0)
        nc.gpsimd.affine_select(
            out=SEL,
            in_=SEL,
            compare_op=Alu.not_equal,
            fill=1.0,
            base=0,
            pattern=[[1, NCAT], [0, CARD]],
            channel_multiplier=-1,
        )

        # tables resident in SBUF: tablesSB[c, f*D + d] = emb[f, c, d]
        tablesSB = cpool.tile([CARD, NCAT * D], f32)
        nc.sync.dma_start(
            out=tablesSB.rearrange("c (f d) -> c f d", d=D),
            in_=emb.rearrange("f c d -> c f d"),
        )

        # fp16 two-term split of the tables: v == hi + lo to ~2^-22 rel
        tbl_hi = cpool.tile([CARD, NCAT * D], f16)
        nc.vector.tensor_copy(out=tbl_hi, in_=tablesSB)
        tbl_hi32 = cpool.tile([CARD, NCAT * D], f32)
        nc.vector.tensor_copy(out=tbl_hi32, in_=tbl_hi)
        tbl_lo32 = cpool.tile([CARD, NCAT * D], f32)
        nc.vector.tensor_tensor(
            out=tbl_lo32, in0=tablesSB, in1=tbl_hi32, op=Alu.subtract
        )
        tbl_lo = cpool.tile([CARD, NCAT * D], f16)
        nc.vector.tensor_copy(out=tbl_lo, in_=tbl_lo32)

        # block-diagonal [W; ones-row bias] matrix: (33, 32*128)
        WB = cpool.tile([NNUM + 1, NNUM * D], f32)
        nc.vector.memset(WB[0:NNUM, :], 0.0)
        nc.sync.dma_start(
            out=WB[NNUM : NNUM + 1, :], in_=bnum.rearrange("f d -> (f d)")
        )
        for f in range(NNUM):
            nc.sync.dma_start(
                out=WB[f : f + 1, f * D : (f + 1) * D], in_=w[f : f + 1, :]
            )

        # whole x shard resident: (128, 8 tiles * 64 feats)
        xall = cpool.tile([P, TILES * NF], f32)
        nc.sync.dma_start(
            out=xall.rearrange("p (t f) -> p t f", f=NF),
            in_=x.rearrange("(t p) f -> p t f", p=P),
        )

        for c in range(CHUNKS):
            # ---- per-tile: idx prep, transposes, numeric ----
            psum_xc = pstpool.tile([NCAT, NB], f32, name="psum_xc", tag="pst", space="PSUM")
            for tl in range(TPC):
                t = c * TPC + tl
                # categorical indices for this tile
                idx_f = wpool.tile([P, NCAT], f32, name="idx_f")
                nc.vector.tensor_scalar(
                    out=idx_f, in0=xall[:, t * NF + 1 : (t + 1) * NF : 2],
                    scalar1=C_RINT, scalar2=C_RINT,
                    op0=Alu.add, op1=Alu.subtract,
                )
                nc.vector.tensor_scalar(
                    out=idx_f, in0=idx_f, scalar1=float(CARD - 1), scalar2=0.0,
                    op0=Alu.min, op1=Alu.max,
                )
                nc.tensor.transpose(
                    out=psum_xc[:, tl * P : (tl + 1) * P],
                    in_=idx_f,
                    identity=identity,
                )

                # numeric: x^T, aug, K=33 matmuls against WB
                psum_xn = pstpool.tile(
                    [NNUM, P], f32, name="psum_xn", tag="pst", space="PSUM"
                )
                nc.tensor.transpose(
                    out=psum_xn,
                    in_=xall[:, t * NF : (t + 1) * NF : 2],
                    identity=identity,
                )
                aug = wpool.tile([NNUM + 1, P], f32, name="aug")
                nc.vector.tensor_copy(out=aug[0:NNUM, :], in_=psum_xn)
                nc.vector.memset(aug[NNUM : NNUM + 1, :], 1.0)

                nbuf = npool.tile([P, NNUM * D], f32, name="nbuf")
                for g in range(NNUM * D // 512):
                    ps = psnpool.tile([P, 512], f32, name="ps", space="PSUM")
                    nc.tensor.matmul(
                        out=ps,
                        lhsT=aug,
                        rhs=WB[:, g * 512 : (g + 1) * 512],
                        start=True,
                        stop=True,
                    )
                    nc.scalar.copy(out=nbuf[:, g * 512 : (g + 1) * 512], in_=ps)
                nc.sync.dma_start(
                    out=out[t * P : (t + 1) * P, 0::2, :],
                    in_=nbuf.rearrange("p (f d) -> p f d", d=D),
                )

            # idx rows for the whole chunk, bf16 (exact for ints < 256)
            xidxT = wpool.tile([NCAT, NB], bf16, name="xidxT")
            nc.vector.tensor_copy(out=xidxT, in_=psum_xc)

            # ---- categorical: one-hot matmul gather ----
            for fg in range(NCAT // 4):
                onehots = []
                for fl in range(4):
                    f = fg * 4 + fl
                    ps_bc = psbpool.tile(
                        [CARD, NB], f32, name="ps_bc", space="PSUM"
                    )
                    nc.tensor.matmul(
                        out=ps_bc,
                        lhsT=SEL[:, f * CARD : (f + 1) * CARD],
                        rhs=xidxT,
                        start=True,
                        stop=True,
                    )
                    oh = ohpool.tile([CARD, NB], f16, name="oh")
                    nc.vector.tensor_scalar(
                        out=oh, in0=ps_bc, scalar1=iota100[0:CARD, :],
                        scalar2=None, op0=Alu.is_equal,
                    )
                    onehots.append(oh)
                for tl in range(TPC):
                    t = c * TPC + tl
                    ps_g = psgpool.tile([P, 512], f32, name="ps_g", space="PSUM")
                    for fl in range(4):
                        f = fg * 4 + fl
                        nc.tensor.matmul(
                            out=ps_g[:, fl * D : (fl + 1) * D],
                            lhsT=onehots[fl][:, tl * P : (tl + 1) * P],
                            rhs=tbl_hi[:, f * D : (f + 1) * D],
                            start=True,
                            stop=False,
                        )
                        nc.tensor.matmul(
                            out=ps_g[:, fl * D : (fl + 1) * D],
                            lhsT=onehots[fl][:, tl * P : (tl + 1) * P],
                            rhs=tbl_lo[:, f * D : (f + 1) * D],
                            start=False,
                            stop=True,
                        )
                    cbuf = cbpool.tile([P, 512], f32, name="cbuf")
                    nc.scalar.copy(out=cbuf, in_=ps_g)
                    nc.sync.dma_start(
                        out=out[
                            t * P : (t + 1) * P, 8 * fg + 1 : 8 * fg + 8 : 2, :
                        ],
                        in_=cbuf.rearrange("p (f d) -> p f d", d=D),
                    )


_NC_CACHE = None


def _build():
    global _NC_CACHE
    if _NC_CACHE is not None:
        return _NC_CACHE
    nc = bacc.Bacc(
        "TRN2", target_bir_lowering=False, debug=False, num_devices=N_CORES
    )
    x = nc.dram_tensor("x", (B_SHARD, NF), f32, kind="ExternalInput").ap()
    w = nc.dram_tensor("W_num", (NNUM, D), f32, kind="ExternalInput").ap()
    bnum = nc.dram_tensor("b_num", (NNUM, D), f32, kind="ExternalInput").ap()
    emb = nc.dram_tensor("emb_tables", (NCAT, CARD, D), f32, kind="ExternalInput").ap()
    out = nc.dram_tensor("out", (B_SHARD, NF, D), f32, kind="ExternalOutput").ap()
    with tile.TileContext(nc) as tc:
        _kernel_body(tc, out, x, w, bnum, emb)
    nc.compile()
    _NC_CACHE = nc
    return nc


def _run(inputs, **kwargs):
    nc = _build()
    x = np.ascontiguousarray(np.asarray(inputs["x"], dtype=np.float32))
    w = np.ascontiguousarray(np.asarray(inputs["W_num"], dtype=np.float32))
    b = np.ascontiguousarray(np.asarray(inputs["b_num"], dtype=np.float32))
    emb = np.ascontiguousarray(np.asarray(inputs["emb_tables"], dtype=np.float32))
    in_maps = [
        {
            "x": np.ascontiguousarray(x[i * B_SHARD : (i + 1) * B_SHARD]),
            "W_num": w,
            "b_num": b,
            "emb_tables": emb,
        }
        for i in range(N_CORES)
    ]
    res = run_bass_kernel_spmd(nc, in_maps, core_ids=list(range(N_CORES)), **kwargs)
    full = np.concatenate([r["out"] for r in res.results], axis=0)
    return full, res


def kernel(x, W_num, b_num, emb_tables):
    full, _ = _run(
        {"x": x, "W_num": W_num, "b_num": b_num, "emb_tables": emb_tables}
    )
    return full


# revision 7
# speedup vs baseline: 2.7942x; 1.0745x over previous
"""MixedFeatureEmbedder Trainium2 kernel (one-hot matmul gather).

Data-parallel over 8 NeuronCores: each core handles 1024 batch rows.

Categorical half (no DMA gather — all PE):
  idx = clip(rint(x_cat), 0, 99) on DVE; PE-transpose idx columns to
  rows; broadcast each feature's idx row across 100 partitions with a
  selector matmul (bf16, exact for small ints); build the one-hot via
  DVE is_equal against the partition index; then out = onehot.T @
  table[f] on PE (fp32) and evacuate PSUM via the scalar engine.

Numeric half: PE transpose of x's even columns + K=33 matmul against a
block-diagonal [W; b] matrix -> x*W + b in PSUM, scalar-engine evac.
"""

import numpy as np

import concourse.bacc as bacc
import concourse.bass as bass
import concourse.mybir as mybir
import concourse.tile as tile
from concourse.bass_utils import run_bass_kernel_spmd
from concourse.masks import make_identity

N_CORES = 8
BATCH = 8192
B_SHARD = BATCH // N_CORES  # 1024
NF = 64
NNUM = 32
NCAT = 32
CARD = 100
D = 128
P = 128
TILES = B_SHARD // P  # 8
TPC = 4  # tiles per chunk
CHUNKS = TILES // TPC  # 2
NB = TPC * P  # batch per chunk = 512
C_RINT = float(2**23)  # (x + 2^23) - 2^23 == rint(x) in f32

f32 = mybir.dt.float32
bf16 = mybir.dt.bfloat16
f16 = mybir.dt.float16
i32 = mybir.dt.int32
Alu = mybir.AluOpType


def _kernel_body(tc, out, x, w, bnum, emb):
    nc = tc.nc

    with (
        tc.tile_pool(name="const", bufs=1) as cpool,
        tc.tile_pool(name="work", bufs=3) as wpool,
        tc.tile_pool(name="oh", bufs=6) as ohpool,
        tc.tile_pool(name="cb", bufs=3) as cbpool,
        tc.tile_pool(name="nbf", bufs=2) as npool,
        tc.tile_pool(name="pst", bufs=2, space="PSUM") as pstpool,
        tc.tile_pool(name="psb", bufs=2, space="PSUM") as psbpool,
        tc.tile_pool(name="psn", bufs=2, space="PSUM") as psnpool,
        tc.tile_pool(name="psg", bufs=2, space="PSUM") as psgpool,
    ):
        # ---- constants ----
        identity = cpool.tile([P, P], f32)
        make_identity(nc, identity)

        # iota100[p, 0] = p (f32) for the one-hot compare
        iota_i = cpool.tile([P, 1], i32)
        nc.gpsimd.iota(iota_i, pattern=[[0, 1]], base=0, channel_multiplier=1)
        iota100 = cpool.tile([P, 1], f32)
        nc.vector.tensor_copy(out=iota100, in_=iota_i)

        # selector: SEL[k, f*CARD + m] = (k == f), bf16
        SEL = cpool.tile([NCAT, NCAT * CARD], bf16)
        nc.gpsimd.memset(SEL, 0.0)
        nc.gpsimd.affine_select(
            out=SEL,
            in_=SEL,
            compare_op=Alu.not_equal,
            fill=1.0,
            base=0,
            pattern=[[1, NCAT], [0, CARD]],
            channel_multiplier=-1,
        )

        # tables resident in SBUF: tablesSB[c, f*D + d] = emb[f, c, d]
        tablesSB = cpool.tile([CARD, NCAT * D], f32)
        nc.sync.dma_start(
            out=tablesSB.rearrange("c (f d) -> c f d", d=D),
            in_=emb.rearrange("f c d -> c f d"),
        )

        # fp16 two-term split of the tables: v == hi + lo to ~2^-22 rel
        tbl_hi = cpool.tile([CARD, NCAT * D], f16)
        nc.vector.tensor_copy(out=tbl_hi, in_=tablesSB)
        tbl_hi32 = cpool.tile([CARD, NCAT * D], f32)
        nc.vector.tensor_copy(out=tbl_hi32, in_=tbl_hi)
        tbl_lo32 = cpool.tile([CARD, NCAT * D], f32)
        nc.vector.tensor_tensor(
            out=tbl_lo32, in0=tablesSB, in1=tbl_hi32, op=Alu.subtract
        )
        tbl_lo = cpool.tile([CARD, NCAT * D], f16)
        nc.vector.tensor_copy(out=tbl_lo, in_=tbl_lo32)

        # block-diagonal [W; ones-row bias] matrix: (33, 32*128)
        WB = cpool.tile([NNUM + 1, NNUM * D], f32)
        nc.vector.memset(WB[0:NNUM, :], 0.0)
        nc.sync.dma_start(
            out=WB[NNUM : NNUM + 1, :], in_=bnum.rearrange("f d -> (f d)")
        )
        for f in range(NNUM):
            nc.sync.dma_start(
                out=WB[f : f + 1, f * D : (f + 1) * D], in_=w[f : f + 1, :]
            )
        WB_hi = cpool.tile([NNUM + 1, NNUM * D], f16)
        nc.vector.tensor_copy(out=WB_hi, in_=WB)
        WB_hi32 = cpool.tile([NNUM + 1, NNUM * D], f32)
        nc.vector.tensor_copy(out=WB_hi32, in_=WB_hi)
        WB_lo32 = cpool.tile([NNUM + 1, NNUM * D], f32)
        nc.vector.tensor_tensor(out=WB_lo32, in0=WB, in1=WB_hi32, op=Alu.subtract)
        WB_lo = cpool.tile([NNUM + 1, NNUM * D], f16)
        nc.vector.tensor_copy(out=WB_lo, in_=WB_lo32)

        # whole x shard resident: (128, 8 tiles * 64 feats)
        xall = cpool.tile([P, TILES * NF], f32)
        nc.sync.dma_start(
            out=xall.rearrange("p (t f) -> p t f", f=NF),
            in_=x.rearrange("(t p) f -> p t f", p=P),
        )

        for c in range(CHUNKS):
            # ---- per-tile: idx prep, transposes, numeric ----
            psum_xc = pstpool.tile([NCAT, NB], f32, name="psum_xc", tag="pst", space="PSUM")
            for tl in range(TPC):
                t = c * TPC + tl
                # categorical indices for this tile
                idx_f = wpool.tile([P, NCAT], f32, name="idx_f")
                nc.vector.tensor_scalar(
                    out=idx_f, in0=xall[:, t * NF + 1 : (t + 1) * NF : 2],
                    scalar1=C_RINT, scalar2=C_RINT,
                    op0=Alu.add, op1=Alu.subtract,
                )
                nc.vector.tensor_scalar(
                    out=idx_f, in0=idx_f, scalar1=float(CARD - 1), scalar2=0.0,
                    op0=Alu.min, op1=Alu.max,
                )
                nc.tensor.transpose(
                    out=psum_xc[:, tl * P : (tl + 1) * P],
                    in_=idx_f,
                    identity=identity,
                )

                # numeric: x^T, aug, K=33 matmuls against WB
                psum_xn = pstpool.tile(
                    [NNUM, P], f32, name="psum_xn", tag="pst", space="PSUM"
                )
                nc.tensor.transpose(
                    out=psum_xn,
                    in_=xall[:, t * NF : (t + 1) * NF : 2],
                    identity=identity,
                )
                aug = wpool.tile([NNUM + 1, P], f32, name="aug")
                nc.vector.tensor_copy(out=aug[0:NNUM, :], in_=psum_xn)
                nc.vector.memset(aug[NNUM : NNUM + 1, :], 1.0)
                aug_hi = wpool.tile([NNUM + 1, P], f16, name="aug_hi")
                nc.vector.tensor_copy(out=aug_hi, in_=aug)
                aug_hi32 = wpool.tile([NNUM + 1, P], f32, name="aug_hi32")
                nc.vector.tensor_copy(out=aug_hi32, in_=aug_hi)
                aug_lo32 = wpool.tile([NNUM + 1, P], f32, name="aug_lo32")
                nc.vector.tensor_tensor(
                    out=aug_lo32, in0=aug, in1=aug_hi32, op=Alu.subtract
                )
                aug_lo = wpool.tile([NNUM + 1, P], f16, name="aug_lo")
                nc.vector.tensor_copy(out=aug_lo, in_=aug_lo32)

                nbuf = npool.tile([P, NNUM * D], f32, name="nbuf")
                for g in range(NNUM * D // 512):
                    ps = psnpool.tile([P, 512], f32, name="ps", space="PSUM")
                    nc.tensor.matmul(
                        out=ps,
                        lhsT=aug_hi,
                        rhs=WB_hi[:, g * 512 : (g + 1) * 512],
                        start=True,
                        stop=False,
                    )
                    nc.tensor.matmul(
                        out=ps,
                        lhsT=aug_hi,
                        rhs=WB_lo[:, g * 512 : (g + 1) * 512],
                        start=False,
                        stop=False,
                    )
                    nc.tensor.matmul(
                        out=ps,
                        lhsT=aug_lo,
                        rhs=WB_hi[:, g * 512 : (g + 1) * 512],
                        start=False,
                        stop=True,
                    )
                    nc.scalar.copy(out=nbuf[:, g * 512 : (g + 1) * 512], in_=ps)
                nc.sync.dma_start(
                    out=out[t * P : (t + 1) * P, 0::2, :],
                    in_=nbuf.rearrange("p (f d) -> p f d", d=D),
                )

            # idx rows for the whole chunk, bf16 (exact for ints < 256)
            xidxT = wpool.tile([NCAT, NB], bf16, name="xidxT")
            nc.vector.tensor_copy(out=xidxT, in_=psum_xc)

            # ---- categorical: one-hot matmul gather ----
            for fg in range(NCAT // 4):
                onehots = []
                for fl in range(4):
                    f = fg * 4 + fl
                    ps_bc = psbpool.tile(
                        [CARD, NB], f32, name="ps_bc", space="PSUM"
                    )
                    nc.tensor.matmul(
                        out=ps_bc,
                        lhsT=SEL[:, f * CARD : (f + 1) * CARD],
                        rhs=xidxT,
                        start=True,
                        stop=True,
                    )
                    oh = ohpool.tile([CARD, NB], f16, name="oh")
                    nc.vector.tensor_scalar(
                        out=oh, in0=ps_bc, scalar1=iota100[0:CARD, :],
                        scalar2=None, op0=Alu.is_equal,
                    )
                    onehots.append(oh)
                for tl in range(TPC):
                    t = c * TPC + tl
                    ps_g = psgpool.tile([P, 512], f32, name="ps_g", space="PSUM")
                    for fl in range(4):
                        f = fg * 4 + fl
                        nc.tensor.matmul(
                            out=ps_g[:, fl * D : (fl + 1) * D],
                            lhsT=onehots[fl][:, tl * P : (tl + 1) * P],
                            rhs=tbl_hi[:, f * D : (f + 1) * D],
                            start=True,
                            stop=False,
                        )
                        nc.tensor.matmul(
                            out=ps_g[:, fl * D : (fl + 1) * D],
                            lhsT=onehots[fl][:, tl * P : (tl + 1) * P],
                            rhs=tbl_lo[:, f * D : (f + 1) * D],
                            start=False,
                            stop=True,
                        )
                    cbuf = cbpool.tile([P, 512], f32, name="cbuf")
                    nc.scalar.copy(out=cbuf, in_=ps_g)
                    nc.sync.dma_start(
                        out=out[
                            t * P : (t + 1) * P, 8 * fg + 1 : 8 * fg + 8 : 2, :
                        ],
                        in_=cbuf.rearrange("p (f d) -> p f d", d=D),
                    )


_NC_CACHE = None


def _build():
    global _NC_CACHE
    if _NC_CACHE is not None:
        return _NC_CACHE
    nc = bacc.Bacc(
        "TRN2", target_bir_lowering=False, debug=False, num_devices=N_CORES
    )
    x = nc.dram_tensor("x", (B_SHARD, NF), f32, kind="ExternalInput").ap()
    w = nc.dram_tensor("W_num", (NNUM, D), f32, kind="ExternalInput").ap()
    bnum = nc.dram_tensor("b_num", (NNUM, D), f32, kind="ExternalInput").ap()
    emb = nc.dram_tensor("emb_tables", (NCAT, CARD, D), f32, kind="ExternalInput").ap()
    out = nc.dram_tensor("out", (B_SHARD, NF, D), f32, kind="ExternalOutput").ap()
    with tile.TileContext(nc) as tc:
        _kernel_body(tc, out, x, w, bnum, emb)
    nc.compile()
    _NC_CACHE = nc
    return nc


def _run(inputs, **kwargs):
    nc = _build()
    x = np.ascontiguousarray(np.asarray(inputs["x"], dtype=np.float32))
    w = np.ascontiguousarray(np.asarray(inputs["W_num"], dtype=np.float32))
    b = np.ascontiguousarray(np.asarray(inputs["b_num"], dtype=np.float32))
    emb = np.ascontiguousarray(np.asarray(inputs["emb_tables"], dtype=np.float32))
    in_maps = [
        {
            "x": np.ascontiguousarray(x[i * B_SHARD : (i + 1) * B_SHARD]),
            "W_num": w,
            "b_num": b,
            "emb_tables": emb,
        }
        for i in range(N_CORES)
    ]
    res = run_bass_kernel_spmd(nc, in_maps, core_ids=list(range(N_CORES)), **kwargs)
    full = np.concatenate([r["out"] for r in res.results], axis=0)
    return full, res


def kernel(x, W_num, b_num, emb_tables):
    full, _ = _run(
        {"x": x, "W_num": W_num, "b_num": b_num, "emb_tables": emb_tables}
    )
    return full
